# revision 15
# baseline (speedup 1.0000x reference)
"""Trainium2 Bass kernel for the DGM (differentiable graph module) problem.

Computation (per graph of n=2048 nodes, B=16 graphs):
  x_aux = relu(x @ W_self + segsum(x @ W_nei, edges)/deg + b)      # GCN layer
  D     = pairwise euclidean distances of x_aux rows (per graph)
  logits = -D + noise
  A     = entmax15(GAMMA * layernorm(logits))                       # rows
  logprobs = log(A + 1e-6)

Sharding: by graph — each of the 8 cores owns 2 graphs (4096 nodes, its
noise/A/logprobs blocks); weights replicated; no cross-core communication.

Device mapping highlights:
  * segment_sum is reformulated as a dense matmul with a per-graph scatter
    matrix STs[src, dst] = count / deg(dst), built host-side from edge_index
    (index preprocessing only; all FLOPs run on the PE).
  * d2 = sq_m + sq_n - 2*x@xT accumulates fully inside PSUM: the main K=64
    matmul plus two K=1 rank-one matmuls ((sq,ones) and (ones,sq)).
  * entmax15 threshold: exact Peters&Martins sort-based algorithm applied to
    the per-row top-8 values (nc.vector.max), which is exact whenever the
    support size is <= 8.  For this problem the support is provably 1 for
    every row with a huge margin (top-2 z-gap ~8.9 vs the 1.0 needed).
  * constants ride in ONE packed DMA and ST streams in 2-chunk
    double-buffered DMAs, keeping per-instruction sync-queue dependency
    fan-in low (HW limits sync wait commands per instruction; bacc's
    generate_event_semaphores splits the rest).
  * notable dead ends (crash at runtime on HW despite passing CoreSim and
    neuronxcc): tensor_tensor_reduce, and mixed bf16/fp32 tensor_tensor
    operands -- both reverted to plain fp32 two-op forms.
"""

import sys
import numpy as np

if "/opt/trn_rl_repo" not in sys.path:
    sys.path.insert(0, "/opt/trn_rl_repo")

B_TOT = 16       # graphs total
NPG = 2048       # nodes per graph
DIN = 128
DE = 64
NCORES = 8
GPC = B_TOT // NCORES   # graphs per core
GAMMA = 5.0
NZBF = False
P = 128          # partitions
CPAD = 265       # packed const columns after xT: Ws(64) Wn(64) b(1) rhoi(8) ident(128)


def build_program(n=NPG, gpc=GPC, topk=True):
    """Emit the SPMD single-core program."""
    import concourse.bass as bass
    import concourse.bacc as bacc
    import concourse.mybir as mybir
    from concourse import tile
    from contextlib import ExitStack

    fp32 = mybir.dt.float32
    Act = mybir.ActivationFunctionType
    Alu = mybir.AluOpType
    Ax = mybir.AxisListType

    nrc = n // P                 # row chunks per graph
    ccw = min(512, n)            # matmul moving free-dim chunk (fp32 max 512)
    ncc = n // ccw
    npc = gpc * n                # nodes per core
    SC = min(2, nrc)             # ST row-chunks per DMA
    nst = nrc // SC              # ST DMAs per graph
    OW, ON, OB, OR, OI = npc, npc + DE, npc + 2 * DE, npc + 2 * DE + 1, npc + 2 * DE + 9

    nc = bacc.Bacc("TRN2", target_bir_lowering=False, debug=False)

    cpk = nc.dram_tensor("cpk", [DIN, npc + CPAD], fp32, kind="ExternalInput").ap()
    STs = nc.dram_tensor("STs", [gpc, n, n], fp32, kind="ExternalInput").ap()
    bf16 = mybir.dt.bfloat16
    NZDT = bf16 if NZBF else fp32
    noise = nc.dram_tensor("noise", [gpc, n, n], NZDT, kind="ExternalInput").ap()

    xaux = nc.dram_tensor("xaux", [npc, DE], fp32, kind="ExternalOutput").ap()
    A = nc.dram_tensor("A", [gpc, n, n], fp32, kind="ExternalOutput").ap()
    LP = nc.dram_tensor("LP", [gpc, n, n], fp32, kind="ExternalOutput").ap()

    with tile.TileContext(nc) as tc, ExitStack() as ctx:
        cpool = ctx.enter_context(tc.tile_pool(name="const", bufs=1))
        pk = cpool.tile([DIN, npc + CPAD], fp32, tag="pk")
        nc.sync.dma_start(pk[:], cpk[:])
        xT_sb = pk[:, 0:npc]
        Ws_sb = pk[:, OW : OW + DE]
        Wn_sb = pk[:, ON : ON + DE]
        b_sb = pk[0:DE, OB : OB + 1]
        crhoi_sb = pk[:, OR : OR + 8]
        ident_sb = pk[0:DE, OI : OI + DE]

        ones64 = cpool.tile([DE, 1], fp32, tag="ones64")
        nc.gpsimd.memset(ones64[:], 1.0)
        czero = cpool.tile([P, 8], fp32, tag="czero")
        nc.gpsimd.memset(czero[:], 0.0)
        cz1 = cpool.tile([P, 1], fp32, tag="cz1")
        nc.gpsimd.memset(cz1[:], 0.0)
        c12 = cpool.tile([P, 1], fp32, tag="c12")
        nc.gpsimd.memset(c12[:], 1e-12)
        c6 = cpool.tile([P, 1], fp32, tag="c6")
        nc.gpsimd.memset(c6[:], 1e-6)
        ones_row = cpool.tile([1, max(512, P)], fp32, tag="onesrow")
        nc.gpsimd.memset(ones_row[:], 1.0)

        # per-graph persistent tiles (reused across graphs; WAW deps are cheap)
        m_sb = cpool.tile([P, nrc * DE], fp32, tag="m")
        RT = cpool.tile([DE, n], fp32, tag="RT")      # x_aux^T
        LT = cpool.tile([DE, n], fp32, tag="LT")      # -2 * x_aux^T
        sq_sb = cpool.tile([1, n], fp32, tag="sq")

        # core-level SBUF pools (never re-zoned -> no DMA-queue dep fanout)
        stp = ctx.enter_context(tc.tile_pool(name="st", bufs=2))
        rmp = ctx.enter_context(tc.tile_pool(name="rm", bufs=2))
        npo = ctx.enter_context(tc.tile_pool(name="nz", bufs=3))
        dpo = ctx.enter_context(tc.tile_pool(name="d2", bufs=3))
        lpo = ctx.enter_context(tc.tile_pool(name="lg", bufs=3))
        upo = ctx.enter_context(tc.tile_pool(name="u", bufs=2))
        opo = ctx.enter_context(tc.tile_pool(name="LP", bufs=2))
        smp = ctx.enter_context(tc.tile_pool(name="sm", bufs=6))

        for g in range(gpc):
            # ---------------- GCN phase ----------------
            with tc.tile_pool(name=f"mps{g}", bufs=2, space="PSUM") as mps:
                for rc in range(nrc):
                    mp = mps.tile([P, DE], fp32, tag="mp")
                    nc.tensor.matmul(
                        mp[:], lhsT=xT_sb[:, g * n + rc * P : g * n + (rc + 1) * P],
                        rhs=Wn_sb, start=True, stop=True)
                    nc.scalar.copy(m_sb[:, rc * DE : (rc + 1) * DE], mp[:])

            with tc.tile_pool(name=f"gps{g}", bufs=1, space="PSUM") as gps:
                xaps = gps.tile([DE, n], fp32, tag="xaps")
                for j in range(ncc):
                    nc.tensor.matmul(
                        xaps[:, j * ccw : (j + 1) * ccw], lhsT=Ws_sb,
                        rhs=xT_sb[:, g * n + j * ccw : g * n + (j + 1) * ccw],
                        start=True, stop=False, skip_group_check=True)
                stv = STs[g].rearrange("(c p) w -> p c w", p=P)
                for sg in range(nst):
                    st = stp.tile([P, SC * n], fp32, tag="st")
                    nc.sync.dma_start(
                        st[:].rearrange("p (c w) -> p c w", c=SC),
                        stv[:, sg * SC : (sg + 1) * SC, :])
                    for ci in range(SC):
                        rc = sg * SC + ci
                        for j in range(ncc):
                            nc.tensor.matmul(
                                xaps[:, j * ccw : (j + 1) * ccw],
                                lhsT=m_sb[:, rc * DE : (rc + 1) * DE],
                                rhs=st[:, ci * n + j * ccw : ci * n + (j + 1) * ccw],
                                start=False, stop=(rc == nrc - 1),
                                skip_group_check=True)
                # x_auxT = relu(h + agg/deg + b)
                nc.scalar.activation(RT[:], xaps[:], Act.Relu, bias=b_sb)
                # squared norms -> sq_row
                nc.scalar.activation(LT[:], RT[:], Act.Square, bias=cz1[0:DE])
                sqps = gps.tile([1, n], fp32, tag="sqps")
                for j in range(ncc):
                    nc.tensor.matmul(
                        sqps[0:1, j * ccw : (j + 1) * ccw], lhsT=ones64[:],
                        rhs=LT[:, j * ccw : (j + 1) * ccw], start=True, stop=True)
                nc.scalar.copy(sq_sb[:], sqps[0:1, :])
                nc.scalar.mul(LT[:], RT[:], -2.0)

            # x_aux row-major output (PE transpose)
            with tc.tile_pool(name=f"tp{g}", bufs=2, space="PSUM") as tps:
                for rc in range(nrc):
                    tp = tps.tile([P, DE], fp32, tag="tp")
                    nc.tensor.transpose(
                        tp[:], RT[:, rc * P : (rc + 1) * P], ident_sb)
                    xa_rm = rmp.tile([P, DE], fp32, tag="xarm")
                    nc.scalar.copy(xa_rm[:], tp[:])
                    nc.sync.dma_start(
                        xaux[g * n + rc * P : g * n + (rc + 1) * P, :], xa_rm[:])

            # ---------------- pairwise + entmax phase ----------------
            with tc.tile_pool(name=f"pp{g}", bufs=2, space="PSUM") as pps:
                for rt in range(nrc):
                    nz = npo.tile([P, n], NZDT, tag="nz")
                    nc.sync.dma_start(nz[:], noise[g, rt * P : (rt + 1) * P, :])

                    pp = pps.tile([P, n], fp32, tag="pp")
                    for j in range(ncc):
                        # d2 = -2*x@xT + sq_m + sq_n, accumulated on the PE
                        nc.tensor.matmul(
                            pp[:, j * ccw : (j + 1) * ccw],
                            lhsT=LT[:, rt * P : (rt + 1) * P],
                            rhs=RT[:, j * ccw : (j + 1) * ccw],
                            start=True, stop=False, skip_group_check=True)
                        nc.tensor.matmul(
                            pp[:, j * ccw : (j + 1) * ccw],
                            lhsT=sq_sb[0:1, rt * P : (rt + 1) * P],
                            rhs=ones_row[0:1, 0:ccw],
                            start=False, stop=False, skip_group_check=True)
                        nc.tensor.matmul(
                            pp[:, j * ccw : (j + 1) * ccw],
                            lhsT=ones_row[0:1, 0:P],
                            rhs=sq_sb[0:1, j * ccw : (j + 1) * ccw],
                            start=False, stop=True, skip_group_check=True)
                    d2c = dpo.tile([P, n], fp32, tag="d2c")
                    nc.vector.tensor_scalar_max(d2c[:], pp[:], 0.0)
                    nc.scalar.activation(d2c[:], d2c[:], Act.Sqrt, bias=c12[:])
                    lg = lpo.tile([P, n], fp32, tag="lg")
                    nc.vector.tensor_tensor(lg[:], nz[:], d2c[:], Alu.subtract)
                    # layernorm stats (two-pass, matches jnp.std); d2c is dead
                    # after the subtract and doubles as scratch output
                    musum = smp.tile([P, 1], fp32, tag="musum")
                    nc.vector.tensor_scalar(
                        d2c[:], lg[:], 1.0, None, Alu.mult, Alu.add,
                        accum_out=musum[:])
                    negmu = smp.tile([P, 1], fp32, tag="negmu")
                    nc.vector.tensor_scalar_mul(negmu[:], musum[:], -1.0 / n)
                    ssq = smp.tile([P, 1], fp32, tag="ssq")
                    nc.scalar.activation(
                        d2c[:], lg[:], Act.Square, bias=negmu[:], accum_out=ssq[:])
                    sd = smp.tile([P, 1], fp32, tag="sd")
                    nc.scalar.activation(sd[:], ssq[:], Act.Sqrt, bias=cz1[:],
                                         scale=1.0 / n)
                    sde = smp.tile([P, 1], fp32, tag="sde")
                    nc.vector.tensor_scalar_add(sde[:], sd[:], 1e-5)
                    rr = smp.tile([P, 1], fp32, tag="rr")
                    nc.vector.reciprocal(rr[:], sde[:])
                    s_t = smp.tile([P, 1], fp32, tag="s_t")
                    nc.vector.tensor_scalar_mul(s_t[:], rr[:], GAMMA / 2.0)
                    c_t = smp.tile([P, 1], fp32, tag="c_t")
                    nc.vector.tensor_tensor(c_t[:], s_t[:], negmu[:], Alu.mult)

                    tau = smp.tile([P, 1], fp32, tag="tau")
                    if topk:
                        # exact entmax threshold from per-row top-8
                        top8 = smp.tile([P, 8], fp32, tag="top8")
                        nc.vector.max(top8[:], lg[:])
                        z8 = smp.tile([P, 8], fp32, tag="z8")
                        nc.scalar.activation(
                            z8[:], top8[:], Act.Identity, bias=c_t[:], scale=s_t[:])
                        z8q = smp.tile([P, 8], fp32, tag="z8q")
                        nc.scalar.activation(z8q[:], z8[:], Act.Square, bias=cz1[:])
                        cs1 = smp.tile([P, 8], fp32, tag="cs1")
                        nc.vector.tensor_tensor_scan(
                            cs1[:], z8[:], czero[:], 0.0, Alu.add, Alu.add)
                        cs2 = smp.tile([P, 8], fp32, tag="cs2")
                        nc.vector.tensor_tensor_scan(
                            cs2[:], z8q[:], czero[:], 0.0, Alu.add, Alu.add)
                        mk = smp.tile([P, 8], fp32, tag="mk")
                        nc.vector.tensor_tensor(mk[:], cs1[:], crhoi_sb, Alu.mult)
                        mq = smp.tile([P, 8], fp32, tag="mq")
                        nc.vector.tensor_tensor(mq[:], cs2[:], crhoi_sb, Alu.mult)
                        mk2 = smp.tile([P, 8], fp32, tag="mk2")
                        nc.vector.tensor_tensor(mk2[:], mk[:], mk[:], Alu.mult)
                        dl = smp.tile([P, 8], fp32, tag="dl")
                        nc.vector.tensor_tensor(dl[:], crhoi_sb, mq[:], Alu.subtract)
                        dl2 = smp.tile([P, 8], fp32, tag="dl2")
                        nc.vector.tensor_tensor(dl2[:], dl[:], mk2[:], Alu.add)
                        dlc = smp.tile([P, 8], fp32, tag="dlc")
                        nc.vector.tensor_scalar_max(dlc[:], dl2[:], 0.0)
                        sq8 = smp.tile([P, 8], fp32, tag="sq8")
                        nc.scalar.activation(sq8[:], dlc[:], Act.Sqrt, bias=cz1[:])
                        tauk = smp.tile([P, 8], fp32, tag="tauk")
                        nc.vector.tensor_tensor(tauk[:], mk[:], sq8[:], Alu.subtract)
                        # tau* = max_k min(tau_k, zs_k): tau_k valid iff
                        # tau_k <= zs_k, and zs_k <= tau* past the support
                        mtau = smp.tile([P, 8], fp32, tag="mtau")
                        nc.vector.tensor_tensor(mtau[:], tauk[:], z8[:], Alu.min)
                        nc.vector.tensor_reduce(tau[:], mtau[:], Ax.X, Alu.max)
                    else:
                        # support==1 closed form: tau = zmax - 1
                        lmax = smp.tile([P, 1], fp32, tag="lmax")
                        nc.vector.tensor_reduce(lmax[:], lg[:], Ax.X, Alu.max)
                        zmax = smp.tile([P, 1], fp32, tag="zmax")
                        nc.scalar.activation(
                            zmax[:], lmax[:], Act.Identity, bias=c_t[:], scale=s_t[:])
                        nc.vector.tensor_scalar_add(tau[:], zmax[:], -1.0)

                    q_t = smp.tile([P, 1], fp32, tag="q_t")
                    nc.vector.tensor_tensor(q_t[:], c_t[:], tau[:], Alu.subtract)

                    u_t = upo.tile([P, n], fp32, tag="u_t")
                    nc.scalar.activation(
                        u_t[:], lg[:], Act.Relu, bias=q_t[:], scale=s_t[:])
                    nc.scalar.activation(u_t[:], u_t[:], Act.Square, bias=cz1[:])
                    nc.sync.dma_start(A[g, rt * P : (rt + 1) * P, :], u_t[:])
                    LP_t = opo.tile([P, n], fp32, tag="LP_t")
                    nc.scalar.activation(LP_t[:], u_t[:], Act.Ln, bias=c6[:])
                    nc.sync.dma_start(LP[g, rt * P : (rt + 1) * P, :], LP_t[:])

    nc.compile()
    return nc


def _prep_inputs(x, edge_index, noise, W_self, W_nei, b, n, gpc, ncores):
    """Host-side sharding + index preprocessing. Returns in_maps list."""
    N = x.shape[0]
    B = N // n
    x = np.ascontiguousarray(np.asarray(x, dtype=np.float32))
    W_self = np.asarray(W_self, dtype=np.float32)
    W_nei = np.asarray(W_nei, dtype=np.float32)
    b = np.asarray(b, dtype=np.float32)
    noise = np.asarray(noise, dtype=np.float32)
    src = np.asarray(edge_index[0]).astype(np.int64)
    dst = np.asarray(edge_index[1]).astype(np.int64)

    deg = np.bincount(dst, minlength=N).astype(np.float32)
    dinv = 1.0 / np.clip(deg, 1.0, None)
    vals = dinv[dst].astype(np.float64)
    flat = (dst // n) * (n * n) + (src % n) * n + (dst % n)
    STs_all = np.bincount(flat, weights=vals, minlength=B * n * n)
    STs_all = STs_all.astype(np.float32).reshape(B, n, n)

    if NZBF:
        import ml_dtypes
        noise_bf = noise.reshape(B, n, n).astype(ml_dtypes.bfloat16)
    else:
        noise_bf = noise.reshape(B, n, n)

    npc = gpc * n
    in_maps = []
    for c in range(ncores):
        lo = c * npc
        pk = np.zeros((DIN, npc + CPAD), np.float32)
        pk[:, 0:npc] = x[lo : lo + npc].T
        pk[:, npc : npc + DE] = W_self
        pk[:, npc + DE : npc + 2 * DE] = W_nei
        pk[0:DE, npc + 2 * DE] = b
        pk[:, npc + 2 * DE + 1 : npc + 2 * DE + 9] = np.tile(
            (1.0 / np.arange(1, 9)).astype(np.float32), (DIN, 1))
        pk[0:DE, npc + 2 * DE + 9 : npc + 2 * DE + 9 + DE] = np.eye(DE, dtype=np.float32)
        in_maps.append({
            "cpk": pk,
            "STs": STs_all[c * gpc : (c + 1) * gpc],
            "noise": noise_bf[c * gpc : (c + 1) * gpc],
        })
    return in_maps


def kernel(x, edge_index, batch, ptr, W_self, W_nei, b, noise, _trace=False):
    from concourse.bass_utils import run_bass_kernel_spmd

    n, gpc, ncores = NPG, GPC, NCORES
    N = np.asarray(x).shape[0]
    B = N // n

    in_maps = _prep_inputs(x, edge_index, noise, W_self, W_nei, b, n, gpc, ncores)
    nc = build_program(n=n, gpc=gpc)
    res = run_bass_kernel_spmd(nc, in_maps, list(range(ncores)), trace=_trace)
    outs = res.results
    x_aux = np.concatenate([outs[c]["xaux"] for c in range(ncores)], axis=0)
    A = np.concatenate([outs[c]["A"] for c in range(ncores)], axis=0).reshape(B, n, n)
    LP = np.concatenate([outs[c]["LP"] for c in range(ncores)], axis=0).reshape(B, n, n)
    if _trace:
        return (x_aux, A, LP), res
    return x_aux, A, LP


# revision 16
# speedup vs baseline: 1.0138x; 1.0138x over previous
"""Trainium2 Bass kernel for the DGM (differentiable graph module) problem.

Computation (per graph of n=2048 nodes, B=16 graphs):
  x_aux = relu(x @ W_self + segsum(x @ W_nei, edges)/deg + b)      # GCN layer
  D     = pairwise euclidean distances of x_aux rows (per graph)
  logits = -D + noise
  A     = entmax15(GAMMA * layernorm(logits))                       # rows
  logprobs = log(A + 1e-6)

Sharding: by graph — each of the 8 cores owns 2 graphs (4096 nodes, its
noise/A/logprobs blocks); weights replicated; no cross-core communication.

Device mapping highlights:
  * segment_sum is reformulated as a dense matmul with a per-graph scatter
    matrix STs[src, dst] = count / deg(dst), built host-side from edge_index
    (index preprocessing only; all FLOPs run on the PE).
  * d2 = sq_m + sq_n - 2*x@xT accumulates fully inside PSUM: the main K=64
    matmul plus two K=1 rank-one matmuls ((sq,ones) and (ones,sq)).
  * entmax15 threshold: exact Peters&Martins sort-based algorithm applied to
    the per-row top-8 values (nc.vector.max), which is exact whenever the
    support size is <= 8.  For this problem the support is provably 1 for
    every row with a huge margin (top-2 z-gap ~8.9 vs the 1.0 needed).
  * constants ride in ONE packed DMA and ST streams in 2-chunk
    double-buffered DMAs, keeping per-instruction sync-queue dependency
    fan-in low (HW limits sync wait commands per instruction; bacc's
    generate_event_semaphores splits the rest).
  * notable dead ends (crash at runtime on HW despite passing CoreSim and
    neuronxcc): tensor_tensor_reduce, and mixed bf16/fp32 tensor_tensor
    operands -- both reverted to plain fp32 two-op forms.
"""

import sys
import numpy as np

if "/opt/trn_rl_repo" not in sys.path:
    sys.path.insert(0, "/opt/trn_rl_repo")

B_TOT = 16       # graphs total
NPG = 2048       # nodes per graph
DIN = 128
DE = 64
NCORES = 8
GPC = B_TOT // NCORES   # graphs per core
GAMMA = 5.0
NZBF = False
P = 128          # partitions
CPAD = 265       # packed const columns after xT: Ws(64) Wn(64) b(1) rhoi(8) ident(128)


def build_program(n=NPG, gpc=GPC, topk=True):
    """Emit the SPMD single-core program."""
    import concourse.bass as bass
    import concourse.bacc as bacc
    import concourse.mybir as mybir
    from concourse import tile
    from contextlib import ExitStack

    fp32 = mybir.dt.float32
    Act = mybir.ActivationFunctionType
    Alu = mybir.AluOpType
    Ax = mybir.AxisListType

    nrc = n // P                 # row chunks per graph
    ccw = min(512, n)            # matmul moving free-dim chunk (fp32 max 512)
    ncc = n // ccw
    npc = gpc * n                # nodes per core
    SC = 1                       # ST row-chunks per DMA
    nst = nrc // SC              # ST DMAs per graph
    OW, ON, OB, OR, OI = npc, npc + DE, npc + 2 * DE, npc + 2 * DE + 1, npc + 2 * DE + 9

    nc = bacc.Bacc("TRN2", target_bir_lowering=False, debug=False)

    cpk = nc.dram_tensor("cpk", [DIN, npc + CPAD], fp32, kind="ExternalInput").ap()
    STs = nc.dram_tensor("STs", [gpc, n, n], fp32, kind="ExternalInput").ap()
    bf16 = mybir.dt.bfloat16
    NZDT = bf16 if NZBF else fp32
    noise = nc.dram_tensor("noise", [gpc, n, n], NZDT, kind="ExternalInput").ap()

    xaux = nc.dram_tensor("xaux", [npc, DE], fp32, kind="ExternalOutput").ap()
    A = nc.dram_tensor("A", [gpc, n, n], fp32, kind="ExternalOutput").ap()
    LP = nc.dram_tensor("LP", [gpc, n, n], fp32, kind="ExternalOutput").ap()

    with tile.TileContext(nc) as tc, ExitStack() as ctx:
        cpool = ctx.enter_context(tc.tile_pool(name="const", bufs=1))
        pk = cpool.tile([DIN, npc + CPAD], fp32, tag="pk")
        nc.sync.dma_start(pk[:], cpk[:])
        xT_sb = pk[:, 0:npc]
        Ws_sb = pk[:, OW : OW + DE]
        Wn_sb = pk[:, ON : ON + DE]
        b_sb = pk[0:DE, OB : OB + 1]
        crhoi_sb = pk[:, OR : OR + 8]
        ident_sb = pk[0:DE, OI : OI + DE]

        ones64 = cpool.tile([DE, 1], fp32, tag="ones64")
        nc.gpsimd.memset(ones64[:], 1.0)
        czero = cpool.tile([P, 8], fp32, tag="czero")
        nc.gpsimd.memset(czero[:], 0.0)
        cz1 = cpool.tile([P, 1], fp32, tag="cz1")
        nc.gpsimd.memset(cz1[:], 0.0)
        c12 = cpool.tile([P, 1], fp32, tag="c12")
        nc.gpsimd.memset(c12[:], 1e-12)
        c6 = cpool.tile([P, 1], fp32, tag="c6")
        nc.gpsimd.memset(c6[:], 1e-6)
        ones_row = cpool.tile([1, max(512, P)], fp32, tag="onesrow")
        nc.gpsimd.memset(ones_row[:], 1.0)

        # per-graph persistent tiles (reused across graphs; WAW deps are cheap)
        m_sb = cpool.tile([P, nrc * DE], fp32, tag="m")
        RT = cpool.tile([DE, n], fp32, tag="RT")      # x_aux^T
        LT = cpool.tile([DE, n], fp32, tag="LT")      # -2 * x_aux^T
        sq_sb = cpool.tile([1, n], fp32, tag="sq")

        # core-level SBUF pools (never re-zoned -> no DMA-queue dep fanout)
        stp = ctx.enter_context(tc.tile_pool(name="st", bufs=4))
        rmp = ctx.enter_context(tc.tile_pool(name="rm", bufs=2))
        npo = ctx.enter_context(tc.tile_pool(name="nz", bufs=3))
        dpo = ctx.enter_context(tc.tile_pool(name="d2", bufs=3))
        lpo = ctx.enter_context(tc.tile_pool(name="lg", bufs=3))
        upo = ctx.enter_context(tc.tile_pool(name="u", bufs=2))
        opo = ctx.enter_context(tc.tile_pool(name="LP", bufs=2))
        smp = ctx.enter_context(tc.tile_pool(name="sm", bufs=6))

        for g in range(gpc):
            # ---------------- GCN phase ----------------
            with tc.tile_pool(name=f"mps{g}", bufs=2, space="PSUM") as mps:
                for rc in range(nrc):
                    mp = mps.tile([P, DE], fp32, tag="mp")
                    nc.tensor.matmul(
                        mp[:], lhsT=xT_sb[:, g * n + rc * P : g * n + (rc + 1) * P],
                        rhs=Wn_sb, start=True, stop=True)
                    nc.scalar.copy(m_sb[:, rc * DE : (rc + 1) * DE], mp[:])

            with tc.tile_pool(name=f"gps{g}", bufs=1, space="PSUM") as gps:
                xaps = gps.tile([DE, n], fp32, tag="xaps")
                for j in range(ncc):
                    nc.tensor.matmul(
                        xaps[:, j * ccw : (j + 1) * ccw], lhsT=Ws_sb,
                        rhs=xT_sb[:, g * n + j * ccw : g * n + (j + 1) * ccw],
                        start=True, stop=False, skip_group_check=True)
                stv = STs[g].rearrange("(c p) w -> p c w", p=P)
                for sg in range(nst):
                    st = stp.tile([P, SC * n], fp32, tag="st")
                    nc.sync.dma_start(
                        st[:].rearrange("p (c w) -> p c w", c=SC),
                        stv[:, sg * SC : (sg + 1) * SC, :])
                    for ci in range(SC):
                        rc = sg * SC + ci
                        for j in range(ncc):
                            nc.tensor.matmul(
                                xaps[:, j * ccw : (j + 1) * ccw],
                                lhsT=m_sb[:, rc * DE : (rc + 1) * DE],
                                rhs=st[:, ci * n + j * ccw : ci * n + (j + 1) * ccw],
                                start=False, stop=(rc == nrc - 1),
                                skip_group_check=True)
                # x_auxT = relu(h + agg/deg + b)
                nc.scalar.activation(RT[:], xaps[:], Act.Relu, bias=b_sb)
                # squared norms -> sq_row
                nc.scalar.activation(LT[:], RT[:], Act.Square, bias=cz1[0:DE])
                sqps = gps.tile([1, n], fp32, tag="sqps")
                for j in range(ncc):
                    nc.tensor.matmul(
                        sqps[0:1, j * ccw : (j + 1) * ccw], lhsT=ones64[:],
                        rhs=LT[:, j * ccw : (j + 1) * ccw], start=True, stop=True)
                nc.scalar.copy(sq_sb[:], sqps[0:1, :])
                nc.scalar.mul(LT[:], RT[:], -2.0)

            # x_aux row-major output (PE transpose)
            with tc.tile_pool(name=f"tp{g}", bufs=2, space="PSUM") as tps:
                for rc in range(nrc):
                    tp = tps.tile([P, DE], fp32, tag="tp")
                    nc.tensor.transpose(
                        tp[:], RT[:, rc * P : (rc + 1) * P], ident_sb)
                    xa_rm = rmp.tile([P, DE], fp32, tag="xarm")
                    nc.scalar.copy(xa_rm[:], tp[:])
                    nc.sync.dma_start(
                        xaux[g * n + rc * P : g * n + (rc + 1) * P, :], xa_rm[:])

            # ---------------- pairwise + entmax phase ----------------
            with tc.tile_pool(name=f"pp{g}", bufs=2, space="PSUM") as pps:
                for rt in range(nrc):
                    nz = npo.tile([P, n], NZDT, tag="nz")
                    nc.sync.dma_start(nz[:], noise[g, rt * P : (rt + 1) * P, :])

                    pp = pps.tile([P, n], fp32, tag="pp")
                    for j in range(ncc):
                        # d2 = -2*x@xT + sq_m + sq_n, accumulated on the PE
                        nc.tensor.matmul(
                            pp[:, j * ccw : (j + 1) * ccw],
                            lhsT=LT[:, rt * P : (rt + 1) * P],
                            rhs=RT[:, j * ccw : (j + 1) * ccw],
                            start=True, stop=False, skip_group_check=True)
                        nc.tensor.matmul(
                            pp[:, j * ccw : (j + 1) * ccw],
                            lhsT=sq_sb[0:1, rt * P : (rt + 1) * P],
                            rhs=ones_row[0:1, 0:ccw],
                            start=False, stop=False, skip_group_check=True)
                        nc.tensor.matmul(
                            pp[:, j * ccw : (j + 1) * ccw],
                            lhsT=ones_row[0:1, 0:P],
                            rhs=sq_sb[0:1, j * ccw : (j + 1) * ccw],
                            start=False, stop=True, skip_group_check=True)
                    d2c = dpo.tile([P, n], fp32, tag="d2c")
                    nc.vector.tensor_scalar_max(d2c[:], pp[:], 0.0)
                    nc.scalar.activation(d2c[:], d2c[:], Act.Sqrt, bias=c12[:])
                    lg = lpo.tile([P, n], fp32, tag="lg")
                    nc.vector.tensor_tensor(lg[:], nz[:], d2c[:], Alu.subtract)
                    # layernorm stats (two-pass, matches jnp.std); d2c is dead
                    # after the subtract and doubles as scratch output
                    musum = smp.tile([P, 1], fp32, tag="musum")
                    nc.vector.tensor_scalar(
                        d2c[:], lg[:], 1.0, None, Alu.mult, Alu.add,
                        accum_out=musum[:])
                    negmu = smp.tile([P, 1], fp32, tag="negmu")
                    nc.vector.tensor_scalar_mul(negmu[:], musum[:], -1.0 / n)
                    ssq = smp.tile([P, 1], fp32, tag="ssq")
                    nc.scalar.activation(
                        d2c[:], lg[:], Act.Square, bias=negmu[:], accum_out=ssq[:])
                    sd = smp.tile([P, 1], fp32, tag="sd")
                    nc.scalar.activation(sd[:], ssq[:], Act.Sqrt, bias=cz1[:],
                                         scale=1.0 / n)
                    sde = smp.tile([P, 1], fp32, tag="sde")
                    nc.vector.tensor_scalar_add(sde[:], sd[:], 1e-5)
                    rr = smp.tile([P, 1], fp32, tag="rr")
                    nc.vector.reciprocal(rr[:], sde[:])
                    s_t = smp.tile([P, 1], fp32, tag="s_t")
                    nc.vector.tensor_scalar_mul(s_t[:], rr[:], GAMMA / 2.0)
                    c_t = smp.tile([P, 1], fp32, tag="c_t")
                    nc.vector.tensor_tensor(c_t[:], s_t[:], negmu[:], Alu.mult)

                    tau = smp.tile([P, 1], fp32, tag="tau")
                    if topk:
                        # exact entmax threshold from per-row top-8
                        top8 = smp.tile([P, 8], fp32, tag="top8")
                        nc.vector.max(top8[:], lg[:])
                        z8 = smp.tile([P, 8], fp32, tag="z8")
                        nc.scalar.activation(
                            z8[:], top8[:], Act.Identity, bias=c_t[:], scale=s_t[:])
                        z8q = smp.tile([P, 8], fp32, tag="z8q")
                        nc.scalar.activation(z8q[:], z8[:], Act.Square, bias=cz1[:])
                        cs1 = smp.tile([P, 8], fp32, tag="cs1")
                        nc.vector.tensor_tensor_scan(
                            cs1[:], z8[:], czero[:], 0.0, Alu.add, Alu.add)
                        cs2 = smp.tile([P, 8], fp32, tag="cs2")
                        nc.vector.tensor_tensor_scan(
                            cs2[:], z8q[:], czero[:], 0.0, Alu.add, Alu.add)
                        mk = smp.tile([P, 8], fp32, tag="mk")
                        nc.vector.tensor_tensor(mk[:], cs1[:], crhoi_sb, Alu.mult)
                        mq = smp.tile([P, 8], fp32, tag="mq")
                        nc.vector.tensor_tensor(mq[:], cs2[:], crhoi_sb, Alu.mult)
                        mk2 = smp.tile([P, 8], fp32, tag="mk2")
                        nc.vector.tensor_tensor(mk2[:], mk[:], mk[:], Alu.mult)
                        dl = smp.tile([P, 8], fp32, tag="dl")
                        nc.vector.tensor_tensor(dl[:], crhoi_sb, mq[:], Alu.subtract)
                        dl2 = smp.tile([P, 8], fp32, tag="dl2")
                        nc.vector.tensor_tensor(dl2[:], dl[:], mk2[:], Alu.add)
                        dlc = smp.tile([P, 8], fp32, tag="dlc")
                        nc.vector.tensor_scalar_max(dlc[:], dl2[:], 0.0)
                        sq8 = smp.tile([P, 8], fp32, tag="sq8")
                        nc.scalar.activation(sq8[:], dlc[:], Act.Sqrt, bias=cz1[:])
                        tauk = smp.tile([P, 8], fp32, tag="tauk")
                        nc.vector.tensor_tensor(tauk[:], mk[:], sq8[:], Alu.subtract)
                        # tau* = max_k min(tau_k, zs_k): tau_k valid iff
                        # tau_k <= zs_k, and zs_k <= tau* past the support
                        mtau = smp.tile([P, 8], fp32, tag="mtau")
                        nc.vector.tensor_tensor(mtau[:], tauk[:], z8[:], Alu.min)
                        nc.vector.tensor_reduce(tau[:], mtau[:], Ax.X, Alu.max)
                    else:
                        # support==1 closed form: tau = zmax - 1
                        lmax = smp.tile([P, 1], fp32, tag="lmax")
                        nc.vector.tensor_reduce(lmax[:], lg[:], Ax.X, Alu.max)
                        zmax = smp.tile([P, 1], fp32, tag="zmax")
                        nc.scalar.activation(
                            zmax[:], lmax[:], Act.Identity, bias=c_t[:], scale=s_t[:])
                        nc.vector.tensor_scalar_add(tau[:], zmax[:], -1.0)

                    q_t = smp.tile([P, 1], fp32, tag="q_t")
                    nc.vector.tensor_tensor(q_t[:], c_t[:], tau[:], Alu.subtract)

                    u_t = upo.tile([P, n], fp32, tag="u_t")
                    nc.scalar.activation(
                        u_t[:], lg[:], Act.Relu, bias=q_t[:], scale=s_t[:])
                    nc.scalar.activation(u_t[:], u_t[:], Act.Square, bias=cz1[:])
                    nc.sync.dma_start(A[g, rt * P : (rt + 1) * P, :], u_t[:])
                    LP_t = opo.tile([P, n], fp32, tag="LP_t")
                    nc.scalar.activation(LP_t[:], u_t[:], Act.Ln, bias=c6[:])
                    nc.sync.dma_start(LP[g, rt * P : (rt + 1) * P, :], LP_t[:])

    nc.compile()
    return nc


def _prep_inputs(x, edge_index, noise, W_self, W_nei, b, n, gpc, ncores):
    """Host-side sharding + index preprocessing. Returns in_maps list."""
    N = x.shape[0]
    B = N // n
    x = np.ascontiguousarray(np.asarray(x, dtype=np.float32))
    W_self = np.asarray(W_self, dtype=np.float32)
    W_nei = np.asarray(W_nei, dtype=np.float32)
    b = np.asarray(b, dtype=np.float32)
    noise = np.asarray(noise, dtype=np.float32)
    src = np.asarray(edge_index[0]).astype(np.int64)
    dst = np.asarray(edge_index[1]).astype(np.int64)

    deg = np.bincount(dst, minlength=N).astype(np.float32)
    dinv = 1.0 / np.clip(deg, 1.0, None)
    vals = dinv[dst].astype(np.float64)
    flat = (dst // n) * (n * n) + (src % n) * n + (dst % n)
    STs_all = np.bincount(flat, weights=vals, minlength=B * n * n)
    STs_all = STs_all.astype(np.float32).reshape(B, n, n)

    if NZBF:
        import ml_dtypes
        noise_bf = noise.reshape(B, n, n).astype(ml_dtypes.bfloat16)
    else:
        noise_bf = noise.reshape(B, n, n)

    npc = gpc * n
    in_maps = []
    for c in range(ncores):
        lo = c * npc
        pk = np.zeros((DIN, npc + CPAD), np.float32)
        pk[:, 0:npc] = x[lo : lo + npc].T
        pk[:, npc : npc + DE] = W_self
        pk[:, npc + DE : npc + 2 * DE] = W_nei
        pk[0:DE, npc + 2 * DE] = b
        pk[:, npc + 2 * DE + 1 : npc + 2 * DE + 9] = np.tile(
            (1.0 / np.arange(1, 9)).astype(np.float32), (DIN, 1))
        pk[0:DE, npc + 2 * DE + 9 : npc + 2 * DE + 9 + DE] = np.eye(DE, dtype=np.float32)
        in_maps.append({
            "cpk": pk,
            "STs": STs_all[c * gpc : (c + 1) * gpc],
            "noise": noise_bf[c * gpc : (c + 1) * gpc],
        })
    return in_maps


def kernel(x, edge_index, batch, ptr, W_self, W_nei, b, noise, _trace=False):
    from concourse.bass_utils import run_bass_kernel_spmd

    n, gpc, ncores = NPG, GPC, NCORES
    N = np.asarray(x).shape[0]
    B = N // n

    in_maps = _prep_inputs(x, edge_index, noise, W_self, W_nei, b, n, gpc, ncores)
    nc = build_program(n=n, gpc=gpc)
    res = run_bass_kernel_spmd(nc, in_maps, list(range(ncores)), trace=_trace)
    outs = res.results
    x_aux = np.concatenate([outs[c]["xaux"] for c in range(ncores)], axis=0)
    A = np.concatenate([outs[c]["A"] for c in range(ncores)], axis=0).reshape(B, n, n)
    LP = np.concatenate([outs[c]["LP"] for c in range(ncores)], axis=0).reshape(B, n, n)
    if _trace:
        return (x_aux, A, LP), res
    return x_aux, A, LP


# revision 17
# speedup vs baseline: 1.0482x; 1.0340x over previous
"""Trainium2 Bass kernel for the DGM (differentiable graph module) problem.

Computation (per graph of n=2048 nodes, B=16 graphs):
  x_aux = relu(x @ W_self + segsum(x @ W_nei, edges)/deg + b)      # GCN layer
  D     = pairwise euclidean distances of x_aux rows (per graph)
  logits = -D + noise
  A     = entmax15(GAMMA * layernorm(logits))                       # rows
  logprobs = log(A + 1e-6)

Sharding: by graph — each of the 8 cores owns 2 graphs (4096 nodes, its
noise/A/logprobs blocks); weights replicated; no cross-core communication.

Device mapping highlights:
  * segment_sum is reformulated as a dense matmul with a per-graph scatter
    matrix STs[src, dst] = count / deg(dst), built host-side from edge_index
    (index preprocessing only; all FLOPs run on the PE).
  * d2 = sq_m + sq_n - 2*x@xT accumulates fully inside PSUM: the main K=64
    matmul plus two K=1 rank-one matmuls ((sq,ones) and (ones,sq)).
  * entmax15 threshold: exact Peters&Martins sort-based algorithm applied to
    the per-row top-8 values (nc.vector.max), which is exact whenever the
    support size is <= 8.  For this problem the support is provably 1 for
    every row with a huge margin (top-2 z-gap ~8.9 vs the 1.0 needed).
  * constants ride in ONE packed DMA and ST streams in 2-chunk
    double-buffered DMAs, keeping per-instruction sync-queue dependency
    fan-in low (HW limits sync wait commands per instruction; bacc's
    generate_event_semaphores splits the rest).
  * notable dead ends (crash at runtime on HW despite passing CoreSim and
    neuronxcc): tensor_tensor_reduce, and mixed bf16/fp32 tensor_tensor
    operands -- both reverted to plain fp32 two-op forms.
"""

import sys
import numpy as np

if "/opt/trn_rl_repo" not in sys.path:
    sys.path.insert(0, "/opt/trn_rl_repo")

B_TOT = 16       # graphs total
NPG = 2048       # nodes per graph
DIN = 128
DE = 64
NCORES = 8
GPC = B_TOT // NCORES   # graphs per core
GAMMA = 5.0
NZBF = False
P = 128          # partitions
CPAD = 265       # packed const columns after xT: Ws(64) Wn(64) b(1) rhoi(8) ident(128)


def build_program(n=NPG, gpc=GPC, topk=True):
    """Emit the SPMD single-core program."""
    import concourse.bass as bass
    import concourse.bacc as bacc
    import concourse.mybir as mybir
    from concourse import tile
    from contextlib import ExitStack

    fp32 = mybir.dt.float32
    Act = mybir.ActivationFunctionType
    Alu = mybir.AluOpType
    Ax = mybir.AxisListType

    nrc = n // P                 # row chunks per graph
    ccw = min(512, n)            # matmul moving free-dim chunk (fp32 max 512)
    ncc = n // ccw
    npc = gpc * n                # nodes per core
    SC = 1                       # ST row-chunks per DMA
    nst = nrc // SC              # ST DMAs per graph
    OW, ON, OB, OR, OI = npc, npc + DE, npc + 2 * DE, npc + 2 * DE + 1, npc + 2 * DE + 9

    nc = bacc.Bacc("TRN2", target_bir_lowering=False, debug=False)

    cpk = nc.dram_tensor("cpk", [DIN, npc + CPAD], fp32, kind="ExternalInput").ap()
    STs = nc.dram_tensor("STs", [gpc, n, n], fp32, kind="ExternalInput").ap()
    bf16 = mybir.dt.bfloat16
    NZDT = bf16 if NZBF else fp32
    noise = nc.dram_tensor("noise", [gpc, n, n], NZDT, kind="ExternalInput").ap()

    xaux = nc.dram_tensor("xaux", [npc, DE], fp32, kind="ExternalOutput").ap()
    A = nc.dram_tensor("A", [gpc, n, n], fp32, kind="ExternalOutput").ap()
    LP = nc.dram_tensor("LP", [gpc, n, n], fp32, kind="ExternalOutput").ap()

    with tile.TileContext(nc) as tc, ExitStack() as ctx:
        cpool = ctx.enter_context(tc.tile_pool(name="const", bufs=1))
        pk = cpool.tile([DIN, npc + CPAD], fp32, tag="pk")
        nc.sync.dma_start(pk[:], cpk[:])
        xT_sb = pk[:, 0:npc]
        Ws_sb = pk[:, OW : OW + DE]
        Wn_sb = pk[:, ON : ON + DE]
        b_sb = pk[0:DE, OB : OB + 1]
        crhoi_sb = pk[:, OR : OR + 8]
        ident_sb = pk[0:DE, OI : OI + DE]

        ones64 = cpool.tile([DE, 1], fp32, tag="ones64")
        nc.gpsimd.memset(ones64[:], 1.0)
        czero = cpool.tile([P, 8], fp32, tag="czero")
        nc.gpsimd.memset(czero[:], 0.0)
        cz1 = cpool.tile([P, 1], fp32, tag="cz1")
        nc.gpsimd.memset(cz1[:], 0.0)
        c12 = cpool.tile([P, 1], fp32, tag="c12")
        nc.gpsimd.memset(c12[:], 1e-12)
        c6 = cpool.tile([P, 1], fp32, tag="c6")
        nc.gpsimd.memset(c6[:], 1e-6)
        ones_row = cpool.tile([1, max(512, P)], fp32, tag="onesrow")
        nc.gpsimd.memset(ones_row[:], 1.0)

        # per-graph persistent tiles (reused across graphs; WAW deps are cheap)
        m_sb = cpool.tile([P, nrc * DE], fp32, tag="m")
        RT = cpool.tile([DE, n], fp32, tag="RT")      # x_aux^T
        LT = cpool.tile([DE, n], fp32, tag="LT")      # -2 * x_aux^T
        sq_sb = cpool.tile([1, n], fp32, tag="sq")

        # core-level SBUF pools (never re-zoned -> no DMA-queue dep fanout)
        stp = ctx.enter_context(tc.tile_pool(name="st", bufs=4))
        rmp = ctx.enter_context(tc.tile_pool(name="rm", bufs=2))
        npo = ctx.enter_context(tc.tile_pool(name="nz", bufs=3))
        dpo = ctx.enter_context(tc.tile_pool(name="d2", bufs=3))
        lpo = ctx.enter_context(tc.tile_pool(name="lg", bufs=3))
        upo = ctx.enter_context(tc.tile_pool(name="u", bufs=2))
        opo = ctx.enter_context(tc.tile_pool(name="LP", bufs=2))
        smp = ctx.enter_context(tc.tile_pool(name="sm", bufs=6))

        for g in range(gpc):
            # ---------------- GCN phase ----------------
            with tc.tile_pool(name=f"mps{g}", bufs=2, space="PSUM") as mps:
                for rc in range(nrc):
                    mp = mps.tile([P, DE], fp32, tag="mp")
                    nc.tensor.matmul(
                        mp[:], lhsT=xT_sb[:, g * n + rc * P : g * n + (rc + 1) * P],
                        rhs=Wn_sb, start=True, stop=True)
                    nc.vector.tensor_copy(m_sb[:, rc * DE : (rc + 1) * DE], mp[:])

            with tc.tile_pool(name=f"gps{g}", bufs=1, space="PSUM") as gps:
                xaps = gps.tile([DE, n], fp32, tag="xaps")
                for j in range(ncc):
                    nc.tensor.matmul(
                        xaps[:, j * ccw : (j + 1) * ccw], lhsT=Ws_sb,
                        rhs=xT_sb[:, g * n + j * ccw : g * n + (j + 1) * ccw],
                        start=True, stop=False, skip_group_check=True)
                stv = STs[g].rearrange("(c p) w -> p c w", p=P)
                for sg in range(nst):
                    st = stp.tile([P, SC * n], fp32, tag="st")
                    nc.sync.dma_start(
                        st[:].rearrange("p (c w) -> p c w", c=SC),
                        stv[:, sg * SC : (sg + 1) * SC, :])
                    for ci in range(SC):
                        rc = sg * SC + ci
                        for j in range(ncc):
                            nc.tensor.matmul(
                                xaps[:, j * ccw : (j + 1) * ccw],
                                lhsT=m_sb[:, rc * DE : (rc + 1) * DE],
                                rhs=st[:, ci * n + j * ccw : ci * n + (j + 1) * ccw],
                                start=False, stop=(rc == nrc - 1),
                                skip_group_check=True)
                # x_auxT = relu(h + agg/deg + b)
                nc.scalar.activation(RT[:], xaps[:], Act.Relu, bias=b_sb)
                # squared norms -> sq_row
                nc.scalar.activation(LT[:], RT[:], Act.Square, bias=cz1[0:DE])
                sqps = gps.tile([1, n], fp32, tag="sqps")
                for j in range(ncc):
                    nc.tensor.matmul(
                        sqps[0:1, j * ccw : (j + 1) * ccw], lhsT=ones64[:],
                        rhs=LT[:, j * ccw : (j + 1) * ccw], start=True, stop=True)
                nc.scalar.copy(sq_sb[:], sqps[0:1, :])
                nc.vector.tensor_scalar_mul(LT[:], RT[:], -2.0)

            # x_aux row-major output (PE transpose)
            with tc.tile_pool(name=f"tp{g}", bufs=2, space="PSUM") as tps:
                for rc in range(nrc):
                    tp = tps.tile([P, DE], fp32, tag="tp")
                    nc.tensor.transpose(
                        tp[:], RT[:, rc * P : (rc + 1) * P], ident_sb)
                    xa_rm = rmp.tile([P, DE], fp32, tag="xarm")
                    nc.vector.tensor_copy(xa_rm[:], tp[:])
                    nc.sync.dma_start(
                        xaux[g * n + rc * P : g * n + (rc + 1) * P, :], xa_rm[:])

            # ---------------- pairwise + entmax phase ----------------
            with tc.tile_pool(name=f"pp{g}", bufs=2, space="PSUM") as pps:
                for rt in range(nrc):
                    nz = npo.tile([P, n], NZDT, tag="nz")
                    nc.sync.dma_start(nz[:], noise[g, rt * P : (rt + 1) * P, :])

                    pp = pps.tile([P, n], fp32, tag="pp")
                    for j in range(ncc):
                        # d2 = -2*x@xT + sq_m + sq_n, accumulated on the PE
                        nc.tensor.matmul(
                            pp[:, j * ccw : (j + 1) * ccw],
                            lhsT=LT[:, rt * P : (rt + 1) * P],
                            rhs=RT[:, j * ccw : (j + 1) * ccw],
                            start=True, stop=False, skip_group_check=True)
                        nc.tensor.matmul(
                            pp[:, j * ccw : (j + 1) * ccw],
                            lhsT=sq_sb[0:1, rt * P : (rt + 1) * P],
                            rhs=ones_row[0:1, 0:ccw],
                            start=False, stop=False, skip_group_check=True)
                        nc.tensor.matmul(
                            pp[:, j * ccw : (j + 1) * ccw],
                            lhsT=ones_row[0:1, 0:P],
                            rhs=sq_sb[0:1, j * ccw : (j + 1) * ccw],
                            start=False, stop=True, skip_group_check=True)
                    d2c = dpo.tile([P, n], fp32, tag="d2c")
                    nc.vector.tensor_scalar_max(d2c[:], pp[:], 0.0)
                    nc.scalar.activation(d2c[:], d2c[:], Act.Sqrt, bias=c12[:])
                    lg = lpo.tile([P, n], fp32, tag="lg")
                    nc.vector.tensor_tensor(lg[:], nz[:], d2c[:], Alu.subtract)
                    # layernorm stats (two-pass, matches jnp.std); d2c is dead
                    # after the subtract and doubles as scratch output
                    musum = smp.tile([P, 1], fp32, tag="musum")
                    nc.vector.tensor_scalar(
                        d2c[:], lg[:], 1.0, None, Alu.mult, Alu.add,
                        accum_out=musum[:])
                    negmu = smp.tile([P, 1], fp32, tag="negmu")
                    nc.vector.tensor_scalar_mul(negmu[:], musum[:], -1.0 / n)
                    ssq = smp.tile([P, 1], fp32, tag="ssq")
                    nc.scalar.activation(
                        d2c[:], lg[:], Act.Square, bias=negmu[:], accum_out=ssq[:])
                    sd = smp.tile([P, 1], fp32, tag="sd")
                    nc.scalar.activation(sd[:], ssq[:], Act.Sqrt, bias=cz1[:],
                                         scale=1.0 / n)
                    sde = smp.tile([P, 1], fp32, tag="sde")
                    nc.vector.tensor_scalar_add(sde[:], sd[:], 1e-5)
                    rr = smp.tile([P, 1], fp32, tag="rr")
                    nc.vector.reciprocal(rr[:], sde[:])
                    s_t = smp.tile([P, 1], fp32, tag="s_t")
                    nc.vector.tensor_scalar_mul(s_t[:], rr[:], GAMMA / 2.0)
                    c_t = smp.tile([P, 1], fp32, tag="c_t")
                    nc.vector.tensor_tensor(c_t[:], s_t[:], negmu[:], Alu.mult)

                    tau = smp.tile([P, 1], fp32, tag="tau")
                    if topk:
                        # exact entmax threshold from per-row top-8
                        top8 = smp.tile([P, 8], fp32, tag="top8")
                        nc.vector.max(top8[:], lg[:])
                        z8 = smp.tile([P, 8], fp32, tag="z8")
                        nc.scalar.activation(
                            z8[:], top8[:], Act.Identity, bias=c_t[:], scale=s_t[:])
                        z8q = smp.tile([P, 8], fp32, tag="z8q")
                        nc.scalar.activation(z8q[:], z8[:], Act.Square, bias=cz1[:])
                        cs1 = smp.tile([P, 8], fp32, tag="cs1")
                        nc.vector.tensor_tensor_scan(
                            cs1[:], z8[:], czero[:], 0.0, Alu.add, Alu.add)
                        cs2 = smp.tile([P, 8], fp32, tag="cs2")
                        nc.vector.tensor_tensor_scan(
                            cs2[:], z8q[:], czero[:], 0.0, Alu.add, Alu.add)
                        mk = smp.tile([P, 8], fp32, tag="mk")
                        nc.vector.tensor_tensor(mk[:], cs1[:], crhoi_sb, Alu.mult)
                        mq = smp.tile([P, 8], fp32, tag="mq")
                        nc.vector.tensor_tensor(mq[:], cs2[:], crhoi_sb, Alu.mult)
                        mk2 = smp.tile([P, 8], fp32, tag="mk2")
                        nc.vector.tensor_tensor(mk2[:], mk[:], mk[:], Alu.mult)
                        dl = smp.tile([P, 8], fp32, tag="dl")
                        nc.vector.tensor_tensor(dl[:], crhoi_sb, mq[:], Alu.subtract)
                        dl2 = smp.tile([P, 8], fp32, tag="dl2")
                        nc.vector.tensor_tensor(dl2[:], dl[:], mk2[:], Alu.add)
                        dlc = smp.tile([P, 8], fp32, tag="dlc")
                        nc.vector.tensor_scalar_max(dlc[:], dl2[:], 0.0)
                        sq8 = smp.tile([P, 8], fp32, tag="sq8")
                        nc.scalar.activation(sq8[:], dlc[:], Act.Sqrt, bias=cz1[:])
                        tauk = smp.tile([P, 8], fp32, tag="tauk")
                        nc.vector.tensor_tensor(tauk[:], mk[:], sq8[:], Alu.subtract)
                        # tau* = max_k min(tau_k, zs_k): tau_k valid iff
                        # tau_k <= zs_k, and zs_k <= tau* past the support
                        mtau = smp.tile([P, 8], fp32, tag="mtau")
                        nc.vector.tensor_tensor(mtau[:], tauk[:], z8[:], Alu.min)
                        nc.vector.tensor_reduce(tau[:], mtau[:], Ax.X, Alu.max)
                    else:
                        # support==1 closed form: tau = zmax - 1
                        lmax = smp.tile([P, 1], fp32, tag="lmax")
                        nc.vector.tensor_reduce(lmax[:], lg[:], Ax.X, Alu.max)
                        zmax = smp.tile([P, 1], fp32, tag="zmax")
                        nc.scalar.activation(
                            zmax[:], lmax[:], Act.Identity, bias=c_t[:], scale=s_t[:])
                        nc.vector.tensor_scalar_add(tau[:], zmax[:], -1.0)

                    q_t = smp.tile([P, 1], fp32, tag="q_t")
                    nc.vector.tensor_tensor(q_t[:], c_t[:], tau[:], Alu.subtract)

                    u_t = upo.tile([P, n], fp32, tag="u_t")
                    nc.scalar.activation(
                        u_t[:], lg[:], Act.Relu, bias=q_t[:], scale=s_t[:])
                    nc.scalar.activation(u_t[:], u_t[:], Act.Square, bias=cz1[:])
                    nc.sync.dma_start(A[g, rt * P : (rt + 1) * P, :], u_t[:])
                    LP_t = opo.tile([P, n], fp32, tag="LP_t")
                    nc.scalar.activation(LP_t[:], u_t[:], Act.Ln, bias=c6[:])
                    nc.sync.dma_start(LP[g, rt * P : (rt + 1) * P, :], LP_t[:])

    nc.compile()
    return nc


def _prep_inputs(x, edge_index, noise, W_self, W_nei, b, n, gpc, ncores):
    """Host-side sharding + index preprocessing. Returns in_maps list."""
    N = x.shape[0]
    B = N // n
    x = np.ascontiguousarray(np.asarray(x, dtype=np.float32))
    W_self = np.asarray(W_self, dtype=np.float32)
    W_nei = np.asarray(W_nei, dtype=np.float32)
    b = np.asarray(b, dtype=np.float32)
    noise = np.asarray(noise, dtype=np.float32)
    src = np.asarray(edge_index[0]).astype(np.int64)
    dst = np.asarray(edge_index[1]).astype(np.int64)

    deg = np.bincount(dst, minlength=N).astype(np.float32)
    dinv = 1.0 / np.clip(deg, 1.0, None)
    vals = dinv[dst].astype(np.float64)
    flat = (dst // n) * (n * n) + (src % n) * n + (dst % n)
    STs_all = np.bincount(flat, weights=vals, minlength=B * n * n)
    STs_all = STs_all.astype(np.float32).reshape(B, n, n)

    if NZBF:
        import ml_dtypes
        noise_bf = noise.reshape(B, n, n).astype(ml_dtypes.bfloat16)
    else:
        noise_bf = noise.reshape(B, n, n)

    npc = gpc * n
    in_maps = []
    for c in range(ncores):
        lo = c * npc
        pk = np.zeros((DIN, npc + CPAD), np.float32)
        pk[:, 0:npc] = x[lo : lo + npc].T
        pk[:, npc : npc + DE] = W_self
        pk[:, npc + DE : npc + 2 * DE] = W_nei
        pk[0:DE, npc + 2 * DE] = b
        pk[:, npc + 2 * DE + 1 : npc + 2 * DE + 9] = np.tile(
            (1.0 / np.arange(1, 9)).astype(np.float32), (DIN, 1))
        pk[0:DE, npc + 2 * DE + 9 : npc + 2 * DE + 9 + DE] = np.eye(DE, dtype=np.float32)
        in_maps.append({
            "cpk": pk,
            "STs": STs_all[c * gpc : (c + 1) * gpc],
            "noise": noise_bf[c * gpc : (c + 1) * gpc],
        })
    return in_maps


def kernel(x, edge_index, batch, ptr, W_self, W_nei, b, noise, _trace=False):
    from concourse.bass_utils import run_bass_kernel_spmd

    n, gpc, ncores = NPG, GPC, NCORES
    N = np.asarray(x).shape[0]
    B = N // n

    in_maps = _prep_inputs(x, edge_index, noise, W_self, W_nei, b, n, gpc, ncores)
    nc = build_program(n=n, gpc=gpc)
    res = run_bass_kernel_spmd(nc, in_maps, list(range(ncores)), trace=_trace)
    outs = res.results
    x_aux = np.concatenate([outs[c]["xaux"] for c in range(ncores)], axis=0)
    A = np.concatenate([outs[c]["A"] for c in range(ncores)], axis=0).reshape(B, n, n)
    LP = np.concatenate([outs[c]["LP"] for c in range(ncores)], axis=0).reshape(B, n, n)
    if _trace:
        return (x_aux, A, LP), res
    return x_aux, A, LP
